# revision 12
# baseline (speedup 1.0000x reference)
# Trainium2 Bass kernel for nn_MEMORY_34986803593776 (scatter_memory).
#
# Math (per sample b):
#   w        = softmax(ck @ mk^T)                             [M]
#   c0       = qa * sigmoid(mem0 @ Wc0 + bc0)                 [DQA]
#   gate     = sigmoid(c0 @ Wm1 + bm1)                        [M*DV]
#   memPre   = mem0 * gate                                    [M*DV]
#   erase    = sig(sig(c0@We+be) + sig(memPre@Wemv+bemv))     [DV]
#   zt       = sig((c0@Wz+bz) + (memPre@Wzmv+bzmv))           [DV]
#   add      = tanh(tanh(zt@Wza+bza) + tanh(memPre@Wamv+bamv))[DV]
#   new      = memPre*(1 - w[m]*erase[dv]) + w[m]*add[dv]     [M,DV]
#
# Sharding: pure data parallel over batch B=16384 across 8 cores (2048/core).
#
# v5 design:
#  - bf16 DRAM I/O (host casts); all DMAs same-dtype -> HWDGE only
#    (loads on sync/SP ring, stores on scalar/ACT ring; gpsimd unused).
#  - Host supplies mem pre-transposed (memT, [f, b] chunks); natural mem is
#    never loaded and no on-chip mem transposes are needed.
#  - gate GEMM runs in the transposed domain (gateT chunks [f,b]);
#    mpreT = memT * gateT elementwise (chunk-pipelined with the gate GEMM
#    and the ez/av GEMMs).
#  - PE transposes mpreT back to natural PSUM chunks (bf16); the combine
#    reads mpre directly from PSUM (bf16 keeps DVE 2x mode) - no copy pass.
#  - 512-sample tiles (4 groups of 128) halve instruction counts.

import os
import numpy as np
import ml_dtypes

ABL = os.environ.get("ABL", "")

B = 16384
M = 64
DV = 64
DK = 64
DQA = 128
F = M * DV  # 4096
N_CORES = 8
B_CORE = B // N_CORES  # 2048
TB = 512                # samples per tile
S = TB // 128           # 4 partition groups per tile
NC = 32                 # f chunks of 128

_BUILD_CACHE = {}


def _build(b_core, iters, with_bm1):
    """Build and compile the single-core Bass program."""
    import concourse.tile as tile
    import concourse.bacc as bacc
    import concourse.mybir as mybir
    from concourse import masks
    from contextlib import ExitStack

    f32 = mybir.dt.float32
    bf16 = mybir.dt.bfloat16
    Alu = mybir.AluOpType
    Act = mybir.ActivationFunctionType

    NT = b_core // TB
    assert b_core % TB == 0

    nc = bacc.Bacc("TRN2", target_bir_lowering=False, debug=False,
                   num_devices=N_CORES)

    # ---- DRAM tensors (host-prepped layouts, all bf16 data) ----
    # Partition-contiguous packing: one descriptor per partition per DMA.
    d_memT = nc.dram_tensor("memT", (NT, 128, NC, TB), bf16, kind="ExternalInput")
    d_qa = nc.dram_tensor("qa", (NT, 128, S, DQA), bf16, kind="ExternalInput")
    d_ck = nc.dram_tensor("ck", (128, NT, S, DK), bf16, kind="ExternalInput")
    d_wc0 = nc.dram_tensor("wc0", (128, NC * 128), bf16, kind="ExternalInput")
    d_wm1 = nc.dram_tensor("wm1", (DQA, F), bf16, kind="ExternalInput")
    d_wez = nc.dram_tensor("wez", (128, NC * 128), bf16, kind="ExternalInput")
    d_wamv = nc.dram_tensor("wamv", (128, NC * 64), bf16, kind="ExternalInput")
    d_wewz = nc.dram_tensor("wewz", (128, 128), bf16, kind="ExternalInput")
    d_wza = nc.dram_tensor("wza", (DV, DV), bf16, kind="ExternalInput")
    d_mkt = nc.dram_tensor("mkt", (DK, M), bf16, kind="ExternalInput")
    d_bias = nc.dram_tensor("biasv", (128, 8), f32, kind="ExternalInput")
    if with_bm1:
        d_bm1 = nc.dram_tensor("bm1r", (1, F), bf16, kind="ExternalInput")
    d_out = nc.dram_tensor("out", (b_core, F), bf16, kind="ExternalOutput")

    out_r = d_out.ap().rearrange("(t s p) f -> t p s f", p=128, s=S)

    with tile.TileContext(nc) as tc:
        with ExitStack() as ctx:
            wpool = ctx.enter_context(tc.tile_pool(name="wpool", bufs=1))
            mpool = ctx.enter_context(tc.tile_pool(name="mpool", bufs=2))
            qpool = ctx.enter_context(tc.tile_pool(name="qpool", bufs=2))
            gpool = ctx.enter_context(tc.tile_pool(name="gpool", bufs=1))
            opool = ctx.enter_context(tc.tile_pool(name="opool", bufs=1))
            tpool = ctx.enter_context(tc.tile_pool(name="tpool", bufs=2))
            spool = ctx.enter_context(tc.tile_pool(name="spool", bufs=2))
            pro = ctx.enter_context(tc.tile_pool(name="pro", bufs=1))
            ps_c0 = ctx.enter_context(tc.tile_pool(name="ps_c0", bufs=1, space="PSUM"))
            ps_sml = ctx.enter_context(tc.tile_pool(name="ps_sml", bufs=1, space="PSUM"))
            ps_gate = ctx.enter_context(tc.tile_pool(name="ps_gate", bufs=2, space="PSUM"))
            ps_ez = ctx.enter_context(tc.tile_pool(name="ps_ez", bufs=1, space="PSUM"))
            ps_tp = ctx.enter_context(tc.tile_pool(name="ps_tp", bufs=3, space="PSUM"))

            # ---- weights into SBUF (once, sync/SP HWDGE ring) ----
            w_c0 = wpool.tile([128, NC, 128], bf16, tag="w_c0")
            nc.sync.dma_start(w_c0[:], d_wc0.ap().rearrange("k (c q) -> k c q", c=NC))
            w_m1 = wpool.tile([128, F], bf16, tag="w_m1")
            nc.sync.dma_start(w_m1[:], d_wm1.ap())
            w_ez = wpool.tile([128, NC, 128], bf16, tag="w_ez")
            nc.sync.dma_start(w_ez[:], d_wez.ap().rearrange("k (c q) -> k c q", c=NC))
            w_amv = wpool.tile([128, NC, 64], bf16, tag="w_amv")
            nc.sync.dma_start(w_amv[:], d_wamv.ap().rearrange("k (c q) -> k c q", c=NC))
            w_ewz = wpool.tile([128, 128], bf16, tag="w_ewz")
            nc.sync.dma_start(w_ewz[:], d_wewz.ap())
            w_za = wpool.tile([DV, DV], bf16, tag="w_za")
            nc.sync.dma_start(w_za[:], d_wza.ap())
            w_mkt = wpool.tile([DK, M], bf16, tag="w_mkt")
            nc.sync.dma_start(w_mkt[:], d_mkt.ap())
            biasv = wpool.tile([128, 8], f32, tag="biasv")
            nc.sync.dma_start(biasv[:], d_bias.ap())
            if with_bm1:
                bm1r = wpool.tile([1, NC, 128], bf16, tag="bm1r")
                nc.sync.dma_start(bm1r[:],
                                  d_bm1.ap().rearrange("o (c q) -> o c q", c=NC))
                ones_b = wpool.tile([1, TB], bf16, tag="ones_b")
                nc.vector.memset(ones_b[:], 1.0)
            ident = wpool.tile([128, 128], bf16, tag="ident")
            masks.make_identity(nc, ident[:])

            bc0 = biasv[:, 0:1]
            b_e = biasv[0:64, 1:2]
            b_z = biasv[0:64, 2:3]
            b_emv = biasv[0:64, 3:4]
            b_zmv = biasv[0:64, 4:5]
            b_amv = biasv[0:64, 5:6]
            b_za = biasv[0:64, 6:7]

            def prologue(w_nat_all, ck_all):
                """Softmax for all tiles: w = softmax(ck @ mk^T), natural [b, m]."""
                for t in range(NT):
                    ck = ck_all[:, t]
                    tk = ps_sml.tile([128, S, 128], bf16, tag="sml")
                    for s in range(S):
                        nc.tensor.transpose(tk[0:64, s, :], ck[:, s, :], ident[:])
                    ckT = spool.tile([64, S, 128], bf16, tag="ckT")
                    nc.vector.tensor_copy(ckT[:], tk[0:64])
                    lg = ps_sml.tile([128, S, 64], f32, tag="sml")
                    for s in range(S):
                        nc.tensor.matmul(lg[:, s], ckT[:, s, :], w_mkt[:],
                                         start=True, stop=True)
                    exv = spool.tile([128, S, 64], f32, tag="exv")
                    sms = spool.tile([128, S], f32, tag="sms")
                    for s in range(S):
                        mx = spool.tile([128, 1], f32, tag="mx")
                        nc.vector.tensor_reduce(mx[:], lg[:, s],
                                                mybir.AxisListType.X,
                                                Alu.max, negate=True)
                        nc.scalar.activation(exv[:, s, :], lg[:, s], Act.Exp,
                                             bias=mx[:])
                        nc.vector.tensor_reduce(sms[:, s:s + 1], exv[:, s, :],
                                                mybir.AxisListType.X, Alu.add)
                    nc.vector.reciprocal(sms[:], sms[:])
                    for s in range(S):
                        nc.vector.tensor_scalar_mul(w_nat_all[:, t, s, :],
                                                    exv[:, s, :], sms[:, s:s + 1])

            def load_tile(t):
                memT = mpool.tile([128, NC, TB], bf16, tag="memT")
                qa = qpool.tile([128, S, DQA], bf16, tag="qa")
                if ABL != "nodma":
                    nc.gpsimd.dma_start(memT[:], d_memT.ap()[t])
                    nc.gpsimd.dma_start(qa[:], d_qa.ap()[t])
                return memT, qa

            def front(t, w_nat_all, loaded):
                """Everything except the final combine: c0, gateT, mpreT,
                ez/av GEMMs, the small epilogue chain, e/a/w natural forms."""
                memT, qa = loaded

                # ---- c0 = sigmoid(mem @ Wc0 + bc0), transposed out [q, b] ----
                c0ps = ps_c0.tile([128, TB], f32, tag="c0")
                for c in range(NC):
                    nc.tensor.matmul(c0ps[:], w_c0[:, c, :], memT[:, c, :],
                                     start=(c == 0), stop=(c == NC - 1))
                c0s = spool.tile([128, TB], bf16, tag="c0s")
                nc.scalar.activation(c0s[:], c0ps[:], Act.Sigmoid, bias=bc0)

                # qaT via PE transposes; multiply straight out of PSUM
                qaT = ps_sml.tile([128, S, 128], bf16, tag="sml")
                for s in range(S):
                    nc.tensor.transpose(qaT[:, s, :], qa[:, s, :], ident[:])
                c0T = spool.tile([128, TB], bf16, tag="c0T")
                nc.vector.tensor_tensor(c0T[:], c0s[:],
                                        qaT[:].rearrange("p s b -> p (s b)"),
                                        op=Alu.mult)

                # ---- wz = [We|Wz]^T @ c0T (small epilogue GEMM, early) ----
                wz = ps_sml.tile([128, TB], f32, tag="sml")
                nc.tensor.matmul(wz[:], w_ewz[:], c0T[:], start=True, stop=True)
                ecT = spool.tile([64, TB], bf16, tag="ecT")
                nc.scalar.activation(ecT[:], wz[0:64], Act.Sigmoid, bias=b_e)
                zc = spool.tile([64, TB], bf16, tag="zc")
                nc.scalar.activation(zc[:], wz[64:128], Act.Identity, bias=b_z)

                # ---- chunk-pipelined gateT -> mpreT -> ez/av GEMMs ----
                gateT = gpool.tile([128, NC, TB], bf16, tag="gateT")
                ezp = ps_ez.tile([128, TB], f32, tag="ez")
                avp = ps_sml.tile([64, TB], f32, tag="sml")
                for c in range(NC):
                    gps = ps_gate.tile([128, TB], f32, tag="g")
                    nc.tensor.matmul(gps[:], w_m1[:, c * 128:(c + 1) * 128],
                                     c0T[:], start=True, stop=not with_bm1)
                    if with_bm1:
                        nc.tensor.matmul(gps[:], bm1r[:, c, :], ones_b[:],
                                         start=False, stop=True)
                    if ABL == "halfact":
                        nc.scalar.activation(gateT[:, c, 0:256], gps[:, 0:256],
                                             Act.Sigmoid)
                    else:
                        nc.scalar.activation(gateT[:, c, :], gps[:], Act.Sigmoid)
                    # mpreT chunk (in-place over memT)
                    nc.vector.tensor_tensor(memT[:, c, :], memT[:, c, :],
                                            gateT[:, c, :], op=Alu.mult)
                    nc.tensor.matmul(ezp[:], w_ez[:, c, :], memT[:, c, :],
                                     start=(c == 0), stop=(c == NC - 1))
                    nc.tensor.matmul(avp[:], w_amv[:, c, :], memT[:, c, :],
                                     start=(c == 0), stop=(c == NC - 1))
                mpreT = memT  # renamed; memT now holds mem * gate (transposed)

                # ---- epilogue chain ([dv, b]) ----
                emvT = spool.tile([64, TB], bf16, tag="emvT")
                nc.scalar.activation(emvT[:], ezp[0:64], Act.Sigmoid, bias=b_emv)
                zmv = spool.tile([64, TB], bf16, tag="zmv")
                nc.scalar.activation(zmv[:], ezp[64:128], Act.Identity, bias=b_zmv)
                amvT = spool.tile([64, TB], bf16, tag="amvT")
                nc.scalar.activation(amvT[:], avp[:], Act.Tanh, bias=b_amv)

                esum = spool.tile([64, TB], bf16, tag="esum")
                nc.vector.tensor_tensor(esum[:], ecT[:], emvT[:], op=Alu.add)
                eT = spool.tile([64, TB], bf16, tag="eT")
                nc.scalar.activation(eT[:], esum[:], Act.Sigmoid)
                zsum = spool.tile([64, TB], bf16, tag="zsum")
                nc.vector.tensor_tensor(zsum[:], zmv[:], zc[:], op=Alu.add)
                ztT = spool.tile([64, TB], bf16, tag="ztT")
                nc.scalar.activation(ztT[:], zsum[:], Act.Sigmoid)
                za = ps_sml.tile([64, TB], f32, tag="sml")
                nc.tensor.matmul(za[:], w_za[:], ztT[:], start=True, stop=True)
                zaT = spool.tile([64, TB], bf16, tag="zaT")
                nc.scalar.activation(zaT[:], za[:], Act.Tanh, bias=b_za)
                asum = spool.tile([64, TB], bf16, tag="asum")
                nc.vector.tensor_tensor(asum[:], zaT[:], amvT[:], op=Alu.add)
                aT = spool.tile([64, TB], bf16, tag="aT")
                nc.scalar.activation(aT[:], asum[:], Act.Tanh)

                # e/a back to natural [b, dv] per group: te[:, g, 0:64]=e, 64:=a
                te = ps_sml.tile([128, S, 128], bf16, tag="sml")
                for g in range(S):
                    nc.tensor.transpose(te[:, g, 0:64],
                                        eT[:, g * 128:(g + 1) * 128],
                                        ident[0:64, 0:64])
                    nc.tensor.transpose(te[:, g, 64:128],
                                        aT[:, g * 128:(g + 1) * 128],
                                        ident[0:64, 0:64])
                ena = spool.tile([128, S, 128], bf16, tag="ena")
                nc.scalar.copy(ena[:], te[:])

                # w pair-duplicated: w2[b, g, 2m, r] = w[b, m]
                w2 = spool.tile([128, S, 128], bf16, tag="w2")
                nc.scalar.copy(
                    w2[:].rearrange("p s (m r) -> p s m r", r=2),
                    w_nat_all[:, t, :, :].unsqueeze(3)
                    .broadcast_to([128, S, 64, 2]))

                return dict(mpreT=mpreT, ena=ena, w2=w2)

            def combine(t, st):
                """new = mpre + w*(a - mpre*e), natural layout, chunked."""
                mpreT, ena, w2 = st["mpreT"], st["ena"], st["w2"]
                out = opool.tile([128, S, F], bf16, tag="out")
                for g in range(S):
                    for j in range(4):
                        tp = ps_tp.tile([128, 1024], bf16, tag="tp")
                        for i in range(1 if ABL == "notp" else 8):
                            c = 8 * j + i
                            nc.tensor.transpose(
                                tp[:, i * 128:(i + 1) * 128],
                                mpreT[:, c, g * 128:(g + 1) * 128], ident[:])
                        tpv = tp[:].rearrange("p (m d) -> p m d", m=16)
                        ebig = (ena[:, g, 0:64].unsqueeze(1)
                                .broadcast_to([128, 16, 64]))
                        abig = (ena[:, g, 64:128].unsqueeze(1)
                                .broadcast_to([128, 16, 64]))
                        tsb = tpool.tile([128, 1024], bf16, tag="tsb")
                        tv = tsb[:].rearrange("p (m d) -> p m d", m=16)
                        nc.vector.tensor_tensor(tv, tpv, ebig, op=Alu.mult)
                        if ABL != "nop23":
                            nc.vector.tensor_tensor(tv, abig, tv, op=Alu.subtract)
                            w4 = (w2[:, g, 32 * j:32 * (j + 1)]
                                  .rearrange("p (m r) -> p m r", r=2)
                                  .unsqueeze(2).broadcast_to([128, 16, 32, 2]))
                            t4 = tsb[:].rearrange("p (m g r) -> p m g r", m=16, r=2)
                            nc.vector.tensor_tensor(t4, t4, w4, op=Alu.mult)
                        nc.vector.tensor_tensor(
                            out[:, g, 1024 * j:1024 * (j + 1)],
                            tp[:], tsb[:], op=Alu.add)
                    # store this group's rows (scalar/ACT HWDGE ring)
                    if ABL != "nodma":
                        nc.gpsimd.dma_start(out_r[t, :, g], out[:, g])

            def whole():
                w_nat_all = pro.tile([128, NT, S, 64], bf16, tag="w_nat_all")
                ck_all = pro.tile([128, NT, S, DK], bf16, tag="ck_all")
                nc.sync.dma_start(ck_all[:], d_ck.ap())
                loaded = load_tile(0)
                prologue(w_nat_all, ck_all)
                st = front(0, w_nat_all, loaded)
                for t in range(NT):
                    st_next = None
                    if t + 1 < NT:
                        nxt = load_tile(t + 1)
                        st_next = front(t + 1, w_nat_all, nxt)
                    combine(t, st)
                    st = st_next

            if iters == 1:
                whole()
            else:
                with tc.For_i(0, iters, 1,
                              hint_engines=(mybir.EngineType.PE,
                                            mybir.EngineType.DVE,
                                            mybir.EngineType.Activation,
                                            mybir.EngineType.Pool,
                                            mybir.EngineType.SP)):
                    whole()

    nc.compile()
    return nc


def _get_nc(b_core, iters, with_bm1):
    key = (b_core, iters, with_bm1, ABL)
    if key not in _BUILD_CACHE:
        _BUILD_CACHE[key] = _build(b_core, iters, with_bm1)
    return _BUILD_CACHE[key]


def _prep_weights(inputs):
    bf = ml_dtypes.bfloat16
    wc0 = np.ascontiguousarray(
        inputs["Wc0"].reshape(32, 128, 128).transpose(1, 0, 2).reshape(128, -1)
    ).astype(bf)
    wez_full = np.concatenate([inputs["Wemv"], inputs["Wzmv"]], axis=1)
    wez = np.ascontiguousarray(
        wez_full.reshape(32, 128, 128).transpose(1, 0, 2).reshape(128, -1)
    ).astype(bf)
    wamv = np.ascontiguousarray(
        inputs["Wamv"].reshape(32, 128, 64).transpose(1, 0, 2).reshape(128, -1)
    ).astype(bf)
    wewz = np.concatenate([inputs["We"], inputs["Wz"]], axis=1).astype(bf)
    wm1 = inputs["Wm1"].astype(bf)
    wza = inputs["Wza"].astype(bf)
    mkt = np.ascontiguousarray(inputs["memory_key"].T).astype(bf)

    biasv = np.zeros((128, 8), np.float32)
    biasv[:, 0] = inputs["bc0"]
    biasv[0:64, 1] = inputs["be"]
    biasv[0:64, 2] = inputs["bz"]
    biasv[0:64, 3] = inputs["bemv"]
    biasv[0:64, 4] = inputs["bzmv"]
    biasv[0:64, 5] = inputs["bamv"]
    biasv[0:64, 6] = inputs["bza"]

    w = dict(wc0=wc0, wm1=wm1, wez=wez, wamv=wamv, wewz=wewz, wza=wza,
             mkt=mkt, biasv=biasv)
    with_bm1 = bool(np.any(inputs["bm1"]))
    if with_bm1:
        w["bm1r"] = inputs["bm1"].reshape(1, F).astype(bf)
    return w, with_bm1


def _make_in_maps(inputs, b_core):
    bf = ml_dtypes.bfloat16
    wdict, _ = _prep_weights(inputs)
    nt = b_core // TB
    # memT[core, t, p, c, b] = mem[core, t*TB + b, 128c + p]
    memT = np.ascontiguousarray(
        inputs["memory_value"].reshape(N_CORES, nt, TB, NC, 128)
        .transpose(0, 1, 4, 3, 2)).astype(bf)
    # qa[core, t, p, s, f] = qa_nat[core, (t*S + s)*128 + p, f]
    qa = np.ascontiguousarray(
        inputs["control_qa"].reshape(N_CORES, nt, S, 128, DQA)
        .transpose(0, 1, 3, 2, 4)).astype(bf)
    # ck[core, p, t, s, f]
    ck = np.ascontiguousarray(
        inputs["control_key"].reshape(N_CORES, nt, S, 128, DK)
        .transpose(0, 3, 1, 2, 4)).astype(bf)
    in_maps = []
    for c in range(N_CORES):
        in_maps.append(dict(memT=memT[c], qa=qa[c], ck=ck[c], **wdict))
    return in_maps


def kernel(**inputs):
    from concourse import bass_utils
    inputs = {k: np.asarray(v) for k, v in inputs.items()}
    _, with_bm1 = _prep_weights(inputs)
    nc = _get_nc(B_CORE, 1, with_bm1)
    in_maps = _make_in_maps(inputs, B_CORE)
    res = bass_utils.run_bass_kernel_spmd(nc, in_maps, core_ids=list(range(N_CORES)))
    out = np.concatenate([r["out"] for r in res.results], axis=0)
    return out.reshape(B, M, DV).astype(np.float32)


# revision 16
# speedup vs baseline: 4.0316x; 4.0316x over previous
# Trainium2 Bass kernel for nn_MEMORY_34986803593776 (scatter_memory).
#
# Math (per sample b):
#   w        = softmax(ck @ mk^T)                             [M]
#   c0       = qa * sigmoid(mem0 @ Wc0 + bc0)                 [DQA]
#   gate     = sigmoid(c0 @ Wm1 + bm1)                        [M*DV]
#   memPre   = mem0 * gate                                    [M*DV]
#   erase    = sig(sig(c0@We+be) + sig(memPre@Wemv+bemv))     [DV]
#   zt       = sig((c0@Wz+bz) + (memPre@Wzmv+bzmv))           [DV]
#   add      = tanh(tanh(zt@Wza+bza) + tanh(memPre@Wamv+bamv))[DV]
#   new      = memPre*(1 - w[m]*erase[dv]) + w[m]*add[dv]     [M,DV]
#
# Sharding: pure data parallel over batch B=16384 across 8 cores (2048/core).
#
# v5 design:
#  - bf16 DRAM I/O (host casts); all DMAs same-dtype -> HWDGE only
#    (loads on sync/SP ring, stores on scalar/ACT ring; gpsimd unused).
#  - Host supplies mem pre-transposed (memT, [f, b] chunks); natural mem is
#    never loaded and no on-chip mem transposes are needed.
#  - gate GEMM runs in the transposed domain (gateT chunks [f,b]);
#    mpreT = memT * gateT elementwise (chunk-pipelined with the gate GEMM
#    and the ez/av GEMMs).
#  - PE transposes mpreT back to natural PSUM chunks (bf16); the combine
#    reads mpre directly from PSUM (bf16 keeps DVE 2x mode) - no copy pass.
#  - 512-sample tiles (4 groups of 128) halve instruction counts.

import os
import numpy as np
import ml_dtypes

def _abl():
    return os.environ.get("ABL", "")

B = 16384
M = 64
DV = 64
DK = 64
DQA = 128
F = M * DV  # 4096
N_CORES = 8
B_CORE = B // N_CORES  # 2048
TB = 512                # samples per tile
S = TB // 128           # 4 partition groups per tile
NC = 32                 # f chunks of 128

_BUILD_CACHE = {}


def _build(b_core, iters, with_bm1):
    """Build and compile the single-core Bass program."""
    import concourse.tile as tile
    import concourse.bacc as bacc
    import concourse.mybir as mybir
    from concourse import masks
    from contextlib import ExitStack

    f32 = mybir.dt.float32
    bf16 = mybir.dt.bfloat16
    Alu = mybir.AluOpType
    Act = mybir.ActivationFunctionType

    NT = b_core // TB
    assert b_core % TB == 0

    nc = bacc.Bacc("TRN2", target_bir_lowering=False, debug=False,
                   num_devices=N_CORES)

    # ---- DRAM tensors (host-prepped layouts, all bf16 data) ----
    # Partition-contiguous packing: one descriptor per partition per DMA.
    d_memT = nc.dram_tensor("memT", (NT, 128, NC, TB), bf16, kind="ExternalInput")
    d_qa = nc.dram_tensor("qa", (NT, 128, S, DQA), bf16, kind="ExternalInput")
    d_ck = nc.dram_tensor("ck", (128, NT, S, DK), bf16, kind="ExternalInput")
    d_wc0 = nc.dram_tensor("wc0", (128, NC * 128), bf16, kind="ExternalInput")
    d_wm1 = nc.dram_tensor("wm1", (DQA, F), bf16, kind="ExternalInput")
    d_wez = nc.dram_tensor("wez", (128, NC * 128), bf16, kind="ExternalInput")
    d_wamv = nc.dram_tensor("wamv", (128, NC * 64), bf16, kind="ExternalInput")
    d_wewz = nc.dram_tensor("wewz", (128, 128), bf16, kind="ExternalInput")
    d_wza = nc.dram_tensor("wza", (DV, DV), bf16, kind="ExternalInput")
    d_mkt = nc.dram_tensor("mkt", (DK, M), bf16, kind="ExternalInput")
    d_bias = nc.dram_tensor("biasv", (128, 8), f32, kind="ExternalInput")
    if with_bm1:
        d_bm1 = nc.dram_tensor("bm1r", (1, F), bf16, kind="ExternalInput")
    d_out = nc.dram_tensor("out", (b_core, F), bf16, kind="ExternalOutput")

    out_r = d_out.ap().rearrange("(t s p) f -> t p s f", p=128, s=S)

    with tile.TileContext(nc) as tc:
        with ExitStack() as ctx:
            wpool = ctx.enter_context(tc.tile_pool(name="wpool", bufs=1))
            mpool = ctx.enter_context(tc.tile_pool(name="mpool", bufs=2))
            qpool = ctx.enter_context(tc.tile_pool(name="qpool", bufs=2))
            gpool = ctx.enter_context(tc.tile_pool(name="gpool", bufs=1))
            opool = ctx.enter_context(tc.tile_pool(name="opool", bufs=1))
            tpool = ctx.enter_context(tc.tile_pool(name="tpool", bufs=2))
            spool = ctx.enter_context(tc.tile_pool(name="spool", bufs=2))
            pro = ctx.enter_context(tc.tile_pool(name="pro", bufs=1))
            ps_c0 = ctx.enter_context(tc.tile_pool(name="ps_c0", bufs=1, space="PSUM"))
            ps_sml = ctx.enter_context(tc.tile_pool(name="ps_sml", bufs=1, space="PSUM"))
            ps_gate = ctx.enter_context(tc.tile_pool(name="ps_gate", bufs=2, space="PSUM"))
            ps_ez = ctx.enter_context(tc.tile_pool(name="ps_ez", bufs=1, space="PSUM"))
            ps_tp = ctx.enter_context(tc.tile_pool(name="ps_tp", bufs=3, space="PSUM"))

            # ---- weights into SBUF (once, sync/SP HWDGE ring) ----
            w_c0 = wpool.tile([128, NC, 128], bf16, tag="w_c0")
            nc.sync.dma_start(w_c0[:], d_wc0.ap().rearrange("k (c q) -> k c q", c=NC))
            w_m1 = wpool.tile([128, F], bf16, tag="w_m1")
            nc.sync.dma_start(w_m1[:], d_wm1.ap())
            w_ez = wpool.tile([128, NC, 128], bf16, tag="w_ez")
            nc.sync.dma_start(w_ez[:], d_wez.ap().rearrange("k (c q) -> k c q", c=NC))
            w_amv = wpool.tile([128, NC, 64], bf16, tag="w_amv")
            nc.sync.dma_start(w_amv[:], d_wamv.ap().rearrange("k (c q) -> k c q", c=NC))
            w_ewz = wpool.tile([128, 128], bf16, tag="w_ewz")
            nc.sync.dma_start(w_ewz[:], d_wewz.ap())
            w_za = wpool.tile([DV, DV], bf16, tag="w_za")
            nc.sync.dma_start(w_za[:], d_wza.ap())
            w_mkt = wpool.tile([DK, M], bf16, tag="w_mkt")
            nc.sync.dma_start(w_mkt[:], d_mkt.ap())
            biasv = wpool.tile([128, 8], f32, tag="biasv")
            nc.sync.dma_start(biasv[:], d_bias.ap())
            if with_bm1:
                bm1r = wpool.tile([1, NC, 128], bf16, tag="bm1r")
                nc.sync.dma_start(bm1r[:],
                                  d_bm1.ap().rearrange("o (c q) -> o c q", c=NC))
                ones_b = wpool.tile([1, TB], bf16, tag="ones_b")
                nc.vector.memset(ones_b[:], 1.0)
            ident = wpool.tile([128, 128], bf16, tag="ident")
            masks.make_identity(nc, ident[:])

            bc0 = biasv[:, 0:1]
            b_e = biasv[0:64, 1:2]
            b_z = biasv[0:64, 2:3]
            b_emv = biasv[0:64, 3:4]
            b_zmv = biasv[0:64, 4:5]
            b_amv = biasv[0:64, 5:6]
            b_za = biasv[0:64, 6:7]

            def prologue(w_nat_all, ck_all):
                """Softmax for all tiles: w = softmax(ck @ mk^T), natural [b, m]."""
                for t in range(NT):
                    ck = ck_all[:, t]
                    tk = ps_sml.tile([128, S, 128], bf16, tag="sml")
                    for s in range(S):
                        nc.tensor.transpose(tk[0:64, s, :], ck[:, s, :], ident[:])
                    ckT = spool.tile([64, S, 128], bf16, tag="ckT")
                    nc.vector.tensor_copy(ckT[:], tk[0:64])
                    lg = ps_sml.tile([128, S, 64], f32, tag="sml")
                    for s in range(S):
                        nc.tensor.matmul(lg[:, s], ckT[:, s, :], w_mkt[:],
                                         start=True, stop=True)
                    exv = spool.tile([128, S, 64], f32, tag="exv")
                    sms = spool.tile([128, S], f32, tag="sms")
                    for s in range(S):
                        mx = spool.tile([128, 1], f32, tag="mx")
                        nc.vector.tensor_reduce(mx[:], lg[:, s],
                                                mybir.AxisListType.X,
                                                Alu.max, negate=True)
                        nc.scalar.activation(exv[:, s, :], lg[:, s], Act.Exp,
                                             bias=mx[:])
                        nc.vector.tensor_reduce(sms[:, s:s + 1], exv[:, s, :],
                                                mybir.AxisListType.X, Alu.add)
                    nc.vector.reciprocal(sms[:], sms[:])
                    for s in range(S):
                        nc.vector.tensor_scalar_mul(w_nat_all[:, t, s, :],
                                                    exv[:, s, :], sms[:, s:s + 1])

            def load_tile(t):
                memT = mpool.tile([128, NC, TB], bf16, tag="memT")
                qa = qpool.tile([128, S, DQA], bf16, tag="qa")
                if _abl() != "nodma":
                    nc.gpsimd.dma_start(memT[:], d_memT.ap()[t])
                    nc.gpsimd.dma_start(qa[:], d_qa.ap()[t])
                return memT, qa

            def front_pre(t, loaded):
                """c0/qaT/c0T and the wz part of the epilogue chain."""
                memT, qa = loaded

                # ---- c0 = sigmoid(mem @ Wc0 + bc0), transposed out [q, b] ----
                c0ps = ps_c0.tile([128, TB], f32, tag="c0")
                for c in range(NC):
                    nc.tensor.matmul(c0ps[:], w_c0[:, c, :], memT[:, c, :],
                                     start=(c == 0), stop=(c == NC - 1))
                c0s = spool.tile([128, TB], bf16, tag="c0s")
                nc.scalar.activation(c0s[:], c0ps[:], Act.Sigmoid, bias=bc0)

                # qaT via PE transposes; multiply straight out of PSUM
                qaT = ps_sml.tile([128, S, 128], bf16, tag="sml")
                for s in range(S):
                    nc.tensor.transpose(qaT[:, s, :], qa[:, s, :], ident[:])
                c0T = spool.tile([128, TB], bf16, tag="c0T")
                nc.vector.tensor_tensor(c0T[:], c0s[:],
                                        qaT[:].rearrange("p s b -> p (s b)"),
                                        op=Alu.mult)

                # ---- wz = [We|Wz]^T @ c0T (small epilogue GEMM, early) ----
                wz = ps_sml.tile([128, TB], f32, tag="sml")
                nc.tensor.matmul(wz[:], w_ewz[:], c0T[:], start=True, stop=True)
                ecT = spool.tile([64, TB], bf16, tag="ecT")
                nc.scalar.activation(ecT[:], wz[0:64], Act.Sigmoid, bias=b_e)
                zc = spool.tile([64, TB], bf16, tag="zc")
                nc.scalar.activation(zc[:], wz[64:128], Act.Identity, bias=b_z)

                gateT = gpool.tile([128, NC, TB], bf16, tag="gateT")
                ezp = ps_ez.tile([128, TB], f32, tag="ez")
                avp = ps_sml.tile([64, TB], f32, tag="sml")
                return dict(memT=memT, gateT=gateT, ezp=ezp, avp=avp,
                            c0T=c0T, ecT=ecT, zc=zc)

            def chunk_step(t, st, c):
                """One f-chunk: gate GEMM + sigmoid + mpreT mult + ez/av GEMM."""
                memT, gateT, ezp, avp, c0T = (st["memT"], st["gateT"],
                                              st["ezp"], st["avp"], st["c0T"])
                gps = ps_gate.tile([128, TB], f32, tag="g")
                nc.tensor.matmul(gps[:], w_m1[:, c * 128:(c + 1) * 128],
                                 c0T[:], start=True, stop=not with_bm1)
                if with_bm1:
                    nc.tensor.matmul(gps[:], bm1r[:, c, :], ones_b[:],
                                     start=False, stop=True)
                nc.scalar.activation(gateT[:, c, :], gps[:], Act.Sigmoid)
                # mpreT chunk (in-place over memT)
                nc.vector.tensor_tensor(memT[:, c, :], memT[:, c, :],
                                        gateT[:, c, :], op=Alu.mult)
                nc.tensor.matmul(ezp[:], w_ez[:, c, :], memT[:, c, :],
                                 start=(c == 0), stop=(c == NC - 1))
                nc.tensor.matmul(avp[:], w_amv[:, c, :], memT[:, c, :],
                                 start=(c == 0), stop=(c == NC - 1))

            def front_post(t, w_nat_all, st):
                ezp, avp, ecT, zc = st["ezp"], st["avp"], st["ecT"], st["zc"]
                mpreT = st["memT"]  # memT now holds mem * gate (transposed)

                # ---- epilogue chain ([dv, b]) ----
                emvT = spool.tile([64, TB], bf16, tag="emvT")
                nc.scalar.activation(emvT[:], ezp[0:64], Act.Sigmoid, bias=b_emv)
                zmv = spool.tile([64, TB], bf16, tag="zmv")
                nc.scalar.activation(zmv[:], ezp[64:128], Act.Identity, bias=b_zmv)
                amvT = spool.tile([64, TB], bf16, tag="amvT")
                nc.scalar.activation(amvT[:], avp[:], Act.Tanh, bias=b_amv)

                esum = spool.tile([64, TB], bf16, tag="esum")
                nc.vector.tensor_tensor(esum[:], ecT[:], emvT[:], op=Alu.add)
                eT = spool.tile([64, TB], bf16, tag="eT")
                nc.scalar.activation(eT[:], esum[:], Act.Sigmoid)
                zsum = spool.tile([64, TB], bf16, tag="zsum")
                nc.vector.tensor_tensor(zsum[:], zmv[:], zc[:], op=Alu.add)
                ztT = spool.tile([64, TB], bf16, tag="ztT")
                nc.scalar.activation(ztT[:], zsum[:], Act.Sigmoid)
                za = ps_sml.tile([64, TB], f32, tag="sml")
                nc.tensor.matmul(za[:], w_za[:], ztT[:], start=True, stop=True)
                zaT = spool.tile([64, TB], bf16, tag="zaT")
                nc.scalar.activation(zaT[:], za[:], Act.Tanh, bias=b_za)
                asum = spool.tile([64, TB], bf16, tag="asum")
                nc.vector.tensor_tensor(asum[:], zaT[:], amvT[:], op=Alu.add)
                aT = spool.tile([64, TB], bf16, tag="aT")
                nc.scalar.activation(aT[:], asum[:], Act.Tanh)

                # e/a back to natural [b, dv] per group: te[:, g, 0:64]=e, 64:=a
                te = ps_sml.tile([128, S, 128], bf16, tag="sml")
                for g in range(S):
                    nc.tensor.transpose(te[:, g, 0:64],
                                        eT[:, g * 128:(g + 1) * 128],
                                        ident[0:64, 0:64])
                    nc.tensor.transpose(te[:, g, 64:128],
                                        aT[:, g * 128:(g + 1) * 128],
                                        ident[0:64, 0:64])
                ena = spool.tile([128, S, 128], bf16, tag="ena")
                nc.scalar.copy(ena[:], te[:])

                # w pair-duplicated: w2[b, g, 2m, r] = w[b, m]
                w2 = spool.tile([128, S, 128], bf16, tag="w2")
                nc.scalar.copy(
                    w2[:].rearrange("p s (m r) -> p s m r", r=2),
                    w_nat_all[:, t, :, :].unsqueeze(3)
                    .broadcast_to([128, S, 64, 2]))

                st["mpreT"] = mpreT
                st["ena"] = ena
                st["w2"] = w2
                return st

            def combine_open(t):
                out = opool.tile([128, S, F], bf16, tag="out")
                return out

            def combine_chunk(t, st, out, k):
                """new = mpre + w*(a - mpre*e) for chunk k (g=k//4, j=k%4)."""
                mpreT, ena, w2 = st["mpreT"], st["ena"], st["w2"]
                g, j = k // 4, k % 4
                if True:
                    if True:
                        tp = ps_tp.tile([128, 1024], bf16, tag="tp")
                        for i in range(1 if _abl() == "notp" else 8):
                            c = 8 * j + i
                            nc.tensor.transpose(
                                tp[:, i * 128:(i + 1) * 128],
                                mpreT[:, c, g * 128:(g + 1) * 128], ident[:])
                        tpv = tp[:].rearrange("p (m d) -> p m d", m=16)
                        ebig = (ena[:, g, 0:64].unsqueeze(1)
                                .broadcast_to([128, 16, 64]))
                        abig = (ena[:, g, 64:128].unsqueeze(1)
                                .broadcast_to([128, 16, 64]))
                        tsb = tpool.tile([128, 1024], bf16, tag="tsb")
                        tv = tsb[:].rearrange("p (m d) -> p m d", m=16)
                        nc.vector.tensor_tensor(tv, tpv, ebig, op=Alu.mult)
                        if _abl() != "nop23":
                            nc.vector.tensor_tensor(tv, abig, tv, op=Alu.subtract)
                            w4 = (w2[:, g, 32 * j:32 * (j + 1)]
                                  .rearrange("p (m r) -> p m r", r=2)
                                  .unsqueeze(2).broadcast_to([128, 16, 32, 2]))
                            t4 = tsb[:].rearrange("p (m g r) -> p m g r", m=16, r=2)
                            nc.vector.tensor_tensor(t4, t4, w4, op=Alu.mult)
                        nc.vector.tensor_tensor(
                            out[:, g, 1024 * j:1024 * (j + 1)],
                            tp[:], tsb[:], op=Alu.add)
                if j == 3 and _abl() != "nodma":
                    # store this group's rows (gpsimd/SWDGE ring)
                    nc.gpsimd.dma_start(out_r[t, :, g], out[:, g])

            def whole():
                w_nat_all = pro.tile([128, NT, S, 64], bf16, tag="w_nat_all")
                ck_all = pro.tile([128, NT, S, DK], bf16, tag="ck_all")
                nc.sync.dma_start(ck_all[:], d_ck.ap())
                loaded = load_tile(0)
                prologue(w_nat_all, ck_all)
                st = front_pre(0, loaded)
                for c in range(NC):
                    chunk_step(0, st, c)
                st = front_post(0, w_nat_all, st)
                # steady state: interleave combine(t) with front(t+1) at
                # chunk granularity so every engine queue has ready work.
                for t in range(NT):
                    out = combine_open(t)
                    if t + 1 < NT:
                        nxt = load_tile(t + 1)
                        # cover the memT DMA latency with combine chunks
                        for k in range(4):
                            combine_chunk(t, st, out, k)
                        st_next = front_pre(t + 1, nxt)
                        done = 0
                        for k in range(4, 16):
                            n = (32 * (k - 3)) // 12 - done
                            for c in range(done, done + n):
                                chunk_step(t + 1, st_next, c)
                            done += n
                            combine_chunk(t, st, out, k)
                        st_next = front_post(t + 1, w_nat_all, st_next)
                    else:
                        for k in range(16):
                            combine_chunk(t, st, out, k)
                        st_next = None
                    st = st_next

            if iters == 1:
                whole()
            else:
                with tc.For_i(0, iters, 1,
                              hint_engines=(mybir.EngineType.PE,
                                            mybir.EngineType.DVE,
                                            mybir.EngineType.Activation,
                                            mybir.EngineType.Pool,
                                            mybir.EngineType.SP)):
                    whole()

    nc.compile()
    return nc


def _get_nc(b_core, iters, with_bm1):
    key = (b_core, iters, with_bm1, _abl())
    if key not in _BUILD_CACHE:
        _BUILD_CACHE[key] = _build(b_core, iters, with_bm1)
    return _BUILD_CACHE[key]


def _prep_weights(inputs):
    bf = ml_dtypes.bfloat16
    wc0 = np.ascontiguousarray(
        inputs["Wc0"].reshape(32, 128, 128).transpose(1, 0, 2).reshape(128, -1)
    ).astype(bf)
    wez_full = np.concatenate([inputs["Wemv"], inputs["Wzmv"]], axis=1)
    wez = np.ascontiguousarray(
        wez_full.reshape(32, 128, 128).transpose(1, 0, 2).reshape(128, -1)
    ).astype(bf)
    wamv = np.ascontiguousarray(
        inputs["Wamv"].reshape(32, 128, 64).transpose(1, 0, 2).reshape(128, -1)
    ).astype(bf)
    wewz = np.concatenate([inputs["We"], inputs["Wz"]], axis=1).astype(bf)
    wm1 = inputs["Wm1"].astype(bf)
    wza = inputs["Wza"].astype(bf)
    mkt = np.ascontiguousarray(inputs["memory_key"].T).astype(bf)

    biasv = np.zeros((128, 8), np.float32)
    biasv[:, 0] = inputs["bc0"]
    biasv[0:64, 1] = inputs["be"]
    biasv[0:64, 2] = inputs["bz"]
    biasv[0:64, 3] = inputs["bemv"]
    biasv[0:64, 4] = inputs["bzmv"]
    biasv[0:64, 5] = inputs["bamv"]
    biasv[0:64, 6] = inputs["bza"]

    w = dict(wc0=wc0, wm1=wm1, wez=wez, wamv=wamv, wewz=wewz, wza=wza,
             mkt=mkt, biasv=biasv)
    with_bm1 = bool(np.any(inputs["bm1"]))
    if with_bm1:
        w["bm1r"] = inputs["bm1"].reshape(1, F).astype(bf)
    return w, with_bm1


def _make_in_maps(inputs, b_core):
    bf = ml_dtypes.bfloat16
    wdict, _ = _prep_weights(inputs)
    nt = b_core // TB
    # memT[core, t, p, c, b] = mem[core, t*TB + b, 128c + p]
    memT = np.ascontiguousarray(
        inputs["memory_value"].reshape(N_CORES, nt, TB, NC, 128)
        .transpose(0, 1, 4, 3, 2)).astype(bf)
    # qa[core, t, p, s, f] = qa_nat[core, (t*S + s)*128 + p, f]
    qa = np.ascontiguousarray(
        inputs["control_qa"].reshape(N_CORES, nt, S, 128, DQA)
        .transpose(0, 1, 3, 2, 4)).astype(bf)
    # ck[core, p, t, s, f]
    ck = np.ascontiguousarray(
        inputs["control_key"].reshape(N_CORES, nt, S, 128, DK)
        .transpose(0, 3, 1, 2, 4)).astype(bf)
    in_maps = []
    for c in range(N_CORES):
        in_maps.append(dict(memT=memT[c], qa=qa[c], ck=ck[c], **wdict))
    return in_maps


def kernel(**inputs):
    from concourse import bass_utils
    inputs = {k: np.asarray(v) for k, v in inputs.items()}
    _, with_bm1 = _prep_weights(inputs)
    nc = _get_nc(B_CORE, 1, with_bm1)
    in_maps = _make_in_maps(inputs, B_CORE)
    res = bass_utils.run_bass_kernel_spmd(nc, in_maps, core_ids=list(range(N_CORES)))
    out = np.concatenate([r["out"] for r in res.results], axis=0)
    return out.reshape(B, M, DV).astype(np.float32)


# revision 19
# speedup vs baseline: 17.7952x; 4.4139x over previous
# Trainium2 Bass kernel for nn_MEMORY_34986803593776 (scatter_memory).
#
# Math (per sample b):
#   w        = softmax(ck @ mk^T)                             [M]
#   c0       = qa * sigmoid(mem0 @ Wc0 + bc0)                 [DQA]
#   gate     = sigmoid(c0 @ Wm1 + bm1)                        [M*DV]
#   memPre   = mem0 * gate                                    [M*DV]
#   erase    = sig(sig(c0@We+be) + sig(memPre@Wemv+bemv))     [DV]
#   zt       = sig((c0@Wz+bz) + (memPre@Wzmv+bzmv))           [DV]
#   add      = tanh(tanh(zt@Wza+bza) + tanh(memPre@Wamv+bamv))[DV]
#   new      = memPre*(1 - w[m]*erase[dv]) + w[m]*add[dv]     [M,DV]
#
# Sharding: pure data parallel over batch B=16384 across 8 cores (2048/core).
#
# v5 design:
#  - bf16 DRAM I/O (host casts); all DMAs same-dtype -> HWDGE only
#    (loads on sync/SP ring, stores on scalar/ACT ring; gpsimd unused).
#  - Host supplies mem pre-transposed (memT, [f, b] chunks); natural mem is
#    never loaded and no on-chip mem transposes are needed.
#  - gate GEMM runs in the transposed domain (gateT chunks [f,b]);
#    mpreT = memT * gateT elementwise (chunk-pipelined with the gate GEMM
#    and the ez/av GEMMs).
#  - PE transposes mpreT back to natural PSUM chunks (bf16); the combine
#    reads mpre directly from PSUM (bf16 keeps DVE 2x mode) - no copy pass.
#  - 512-sample tiles (4 groups of 128) halve instruction counts.

import os
import numpy as np
import ml_dtypes

def _abl():
    return os.environ.get("ABL", "")

B = 16384
M = 64
DV = 64
DK = 64
DQA = 128
F = M * DV  # 4096
N_CORES = 8
B_CORE = B // N_CORES  # 2048
TB = 512                # samples per tile
S = TB // 128           # 4 partition groups per tile
NC = 32                 # f chunks of 128

_BUILD_CACHE = {}


def _build(b_core, iters, with_bm1):
    """Build and compile the single-core Bass program."""
    import concourse.tile as tile
    import concourse.bacc as bacc
    import concourse.mybir as mybir
    from concourse import masks
    from contextlib import ExitStack

    f32 = mybir.dt.float32
    bf16 = mybir.dt.bfloat16
    Alu = mybir.AluOpType
    Act = mybir.ActivationFunctionType

    NT = b_core // TB
    assert b_core % TB == 0

    nc = bacc.Bacc("TRN2", target_bir_lowering=False, debug=False,
                   num_devices=N_CORES)

    # ---- DRAM tensors (host-prepped layouts, all bf16 data) ----
    # Partition-contiguous packing: one descriptor per partition per DMA.
    d_memT = nc.dram_tensor("memT", (NT, 128, NC, TB), bf16, kind="ExternalInput")
    d_qa = nc.dram_tensor("qa", (NT, 128, S, DQA), bf16, kind="ExternalInput")
    d_ck = nc.dram_tensor("ck", (128, NT, S, DK), bf16, kind="ExternalInput")
    d_wc0 = nc.dram_tensor("wc0", (128, NC * 128), bf16, kind="ExternalInput")
    d_wm1 = nc.dram_tensor("wm1", (DQA, F), bf16, kind="ExternalInput")
    d_wez = nc.dram_tensor("wez", (128, NC * 128), bf16, kind="ExternalInput")
    d_wamv = nc.dram_tensor("wamv", (128, NC * 64), bf16, kind="ExternalInput")
    d_wewz = nc.dram_tensor("wewz", (128, 128), bf16, kind="ExternalInput")
    d_wza = nc.dram_tensor("wza", (DV, DV), bf16, kind="ExternalInput")
    d_mkt = nc.dram_tensor("mkt", (DK, M), bf16, kind="ExternalInput")
    d_bias = nc.dram_tensor("biasv", (128, 8), f32, kind="ExternalInput")
    if with_bm1:
        d_bm1 = nc.dram_tensor("bm1r", (1, F), bf16, kind="ExternalInput")
    d_out = nc.dram_tensor("out", (b_core, F), bf16, kind="ExternalOutput")

    out_r = d_out.ap().rearrange("(t s p) f -> t p s f", p=128, s=S)

    with tile.TileContext(nc) as tc:
        with ExitStack() as ctx:
            wpool = ctx.enter_context(tc.tile_pool(name="wpool", bufs=1))
            mpool = ctx.enter_context(tc.tile_pool(name="mpool", bufs=2))
            qpool = ctx.enter_context(tc.tile_pool(name="qpool", bufs=2))
            gpool = ctx.enter_context(tc.tile_pool(name="gpool", bufs=1))
            opool = ctx.enter_context(tc.tile_pool(name="opool", bufs=1))
            tpool = ctx.enter_context(tc.tile_pool(name="tpool", bufs=2))
            spool = ctx.enter_context(tc.tile_pool(name="spool", bufs=2))
            pro = ctx.enter_context(tc.tile_pool(name="pro", bufs=1))
            ps_acc = ctx.enter_context(tc.tile_pool(name="ps_acc", bufs=1, space="PSUM"))
            ps_sml = ctx.enter_context(tc.tile_pool(name="ps_sml", bufs=1, space="PSUM"))
            ps_gate = ctx.enter_context(tc.tile_pool(name="ps_gate", bufs=1, space="PSUM"))
            ps_tp = ctx.enter_context(tc.tile_pool(name="ps_tp", bufs=2, space="PSUM"))

            # ---- weights into SBUF (once, sync/SP HWDGE ring) ----
            w_c0 = wpool.tile([128, NC, 128], bf16, tag="w_c0")
            nc.sync.dma_start(w_c0[:], d_wc0.ap().rearrange("k (c q) -> k c q", c=NC))
            w_m1 = wpool.tile([128, F], bf16, tag="w_m1")
            nc.sync.dma_start(w_m1[:], d_wm1.ap())
            w_ez = wpool.tile([128, NC, 128], bf16, tag="w_ez")
            nc.sync.dma_start(w_ez[:], d_wez.ap().rearrange("k (c q) -> k c q", c=NC))
            w_amv = wpool.tile([128, NC, 64], bf16, tag="w_amv")
            nc.sync.dma_start(w_amv[:], d_wamv.ap().rearrange("k (c q) -> k c q", c=NC))
            w_ewz = wpool.tile([128, 128], bf16, tag="w_ewz")
            nc.sync.dma_start(w_ewz[:], d_wewz.ap())
            w_za = wpool.tile([DV, DV], bf16, tag="w_za")
            nc.sync.dma_start(w_za[:], d_wza.ap())
            w_mkt = wpool.tile([DK, M], bf16, tag="w_mkt")
            nc.sync.dma_start(w_mkt[:], d_mkt.ap())
            biasv = wpool.tile([128, 8], f32, tag="biasv")
            nc.sync.dma_start(biasv[:], d_bias.ap())
            if with_bm1:
                bm1r = wpool.tile([1, NC, 128], bf16, tag="bm1r")
                nc.sync.dma_start(bm1r[:],
                                  d_bm1.ap().rearrange("o (c q) -> o c q", c=NC))
                ones_b = wpool.tile([1, TB], bf16, tag="ones_b")
                nc.vector.memset(ones_b[:], 1.0)
            ident = wpool.tile([128, 128], bf16, tag="ident")
            masks.make_identity(nc, ident[:])

            bc0 = biasv[:, 0:1]
            b_e = biasv[0:64, 1:2]
            b_z = biasv[0:64, 2:3]
            b_emv = biasv[0:64, 3:4]
            b_zmv = biasv[0:64, 4:5]
            b_amv = biasv[0:64, 5:6]
            b_za = biasv[0:64, 6:7]

            def prologue(w_nat_all, ck_all):
                """Softmax for all tiles: w = softmax(ck @ mk^T), natural [b, m]."""
                for t in range(NT):
                    ck = ck_all[:, t]
                    tk = ps_sml.tile([128, S, 128], bf16, tag="sml")
                    for s in range(S):
                        nc.tensor.transpose(tk[0:64, s, :], ck[:, s, :], ident[:])
                    ckT = spool.tile([64, S, 128], bf16, tag="ckT")
                    nc.vector.tensor_copy(ckT[:], tk[0:64])
                    lg = ps_sml.tile([128, S, 64], f32, tag="sml")
                    for s in range(S):
                        nc.tensor.matmul(lg[:, s], ckT[:, s, :], w_mkt[:],
                                         start=True, stop=True)
                    exv = spool.tile([128, S, 64], f32, tag="exv")
                    sms = spool.tile([128, S], f32, tag="sms")
                    for s in range(S):
                        mx = spool.tile([128, 1], f32, tag="mx")
                        nc.vector.tensor_reduce(mx[:], lg[:, s],
                                                mybir.AxisListType.X,
                                                Alu.max, negate=True)
                        nc.scalar.activation(exv[:, s, :], lg[:, s], Act.Exp,
                                             bias=mx[:])
                        nc.vector.tensor_reduce(sms[:, s:s + 1], exv[:, s, :],
                                                mybir.AxisListType.X, Alu.add)
                    nc.vector.reciprocal(sms[:], sms[:])
                    for s in range(S):
                        nc.vector.tensor_scalar_mul(w_nat_all[:, t, s, :],
                                                    exv[:, s, :], sms[:, s:s + 1])

            def load_tile(t):
                memT = mpool.tile([128, NC, TB], bf16, tag="memT")
                qa = qpool.tile([128, S, DQA], bf16, tag="qa")
                eng = nc.sync if _abl() == "hwdge" else nc.gpsimd
                if _abl() != "nodma":
                    eng.dma_start(memT[:], d_memT.ap()[t])
                    eng.dma_start(qa[:], d_qa.ap()[t])
                return memT, qa

            def front_pre(t, loaded):
                """c0/qaT/c0T and the wz part of the epilogue chain."""
                memT, qa = loaded

                # ---- c0 = sigmoid(mem @ Wc0 + bc0), transposed out [q, b] ----
                c0ps = ps_acc.tile([128, TB], f32, tag="acc")
                for c in range(NC):
                    nc.tensor.matmul(c0ps[:], w_c0[:, c, :], memT[:, c, :],
                                     start=(c == 0), stop=(c == NC - 1))
                c0s = spool.tile([128, TB], bf16, tag="c0s")
                nc.scalar.activation(c0s[:], c0ps[:], Act.Sigmoid, bias=bc0)

                # qaT via PE transposes; multiply straight out of PSUM
                qaT = ps_sml.tile([128, S, 128], bf16, tag="sml")
                for s in range(S):
                    nc.tensor.transpose(qaT[:, s, :], qa[:, s, :], ident[:])
                c0T = spool.tile([128, TB], bf16, tag="c0T")
                nc.vector.tensor_tensor(c0T[:], c0s[:],
                                        qaT[:].rearrange("p s b -> p (s b)"),
                                        op=Alu.mult)
                del c0ps

                # ---- wz = [We|Wz]^T @ c0T (small epilogue GEMM, early) ----
                wz = ps_sml.tile([128, TB], f32, tag="sml")
                nc.tensor.matmul(wz[:], w_ewz[:], c0T[:], start=True, stop=True)
                ecT = spool.tile([64, TB], bf16, tag="ecT")
                nc.scalar.activation(ecT[:], wz[0:64], Act.Sigmoid, bias=b_e)
                zc = spool.tile([64, TB], bf16, tag="zc")
                nc.scalar.activation(zc[:], wz[64:128], Act.Identity, bias=b_z)

                gateT = gpool.tile([128, NC, TB], bf16, tag="gateT")
                ezp = ps_acc.tile([128, TB], f32, tag="acc")
                avp = ps_sml.tile([64, TB], f32, tag="sml")
                return dict(memT=memT, gateT=gateT, ezp=ezp, avp=avp,
                            c0T=c0T, ecT=ecT, zc=zc)

            def chunk_step(t, st, cc):
                """One f-chunk pair (2cc, 2cc+1): gate GEMMs + one sigmoid +
                one mpreT mult + ez/av GEMMs."""
                memT, gateT, ezp, avp, c0T = (st["memT"], st["gateT"],
                                              st["ezp"], st["avp"], st["c0T"])
                c0, c1 = 2 * cc, 2 * cc + 1
                gps = ps_gate.tile([128, 2, TB], f32, tag="g")
                for i, c in enumerate((c0, c1)):
                    nc.tensor.matmul(gps[:, i], w_m1[:, c * 128:(c + 1) * 128],
                                     c0T[:], start=True, stop=not with_bm1)
                    if with_bm1:
                        nc.tensor.matmul(gps[:, i], bm1r[:, c, :], ones_b[:],
                                         start=False, stop=True)
                nc.scalar.activation(gateT[:, c0:c1 + 1, :], gps[:], Act.Sigmoid)
                # mpreT pair (in-place over memT)
                nc.vector.tensor_tensor(memT[:, c0:c1 + 1, :],
                                        memT[:, c0:c1 + 1, :],
                                        gateT[:, c0:c1 + 1, :], op=Alu.mult)
                for c in (c0, c1):
                    nc.tensor.matmul(ezp[:], w_ez[:, c, :], memT[:, c, :],
                                     start=(c == 0), stop=(c == NC - 1))
                    nc.tensor.matmul(avp[:], w_amv[:, c, :], memT[:, c, :],
                                     start=(c == 0), stop=(c == NC - 1))

            def front_post(t, w_nat_all, st):
                ezp, avp, ecT, zc = st["ezp"], st["avp"], st["ecT"], st["zc"]
                mpreT = st["memT"]  # memT now holds mem * gate (transposed)

                # ---- epilogue chain ([dv, b]) ----
                emvT = spool.tile([64, TB], bf16, tag="emvT")
                nc.scalar.activation(emvT[:], ezp[0:64], Act.Sigmoid, bias=b_emv)
                zmv = spool.tile([64, TB], bf16, tag="zmv")
                nc.scalar.activation(zmv[:], ezp[64:128], Act.Identity, bias=b_zmv)
                amvT = spool.tile([64, TB], bf16, tag="amvT")
                nc.scalar.activation(amvT[:], avp[:], Act.Tanh, bias=b_amv)

                esum = spool.tile([64, TB], bf16, tag="esum")
                nc.gpsimd.tensor_tensor(esum[:], ecT[:], emvT[:], op=Alu.add)
                eT = spool.tile([64, TB], bf16, tag="eT")
                nc.scalar.activation(eT[:], esum[:], Act.Sigmoid)
                zsum = spool.tile([64, TB], bf16, tag="zsum")
                nc.gpsimd.tensor_tensor(zsum[:], zmv[:], zc[:], op=Alu.add)
                ztT = spool.tile([64, TB], bf16, tag="ztT")
                nc.scalar.activation(ztT[:], zsum[:], Act.Sigmoid)
                za = ps_sml.tile([64, TB], f32, tag="sml")
                nc.tensor.matmul(za[:], w_za[:], ztT[:], start=True, stop=True)
                zaT = spool.tile([64, TB], bf16, tag="zaT")
                nc.scalar.activation(zaT[:], za[:], Act.Tanh, bias=b_za)
                asum = spool.tile([64, TB], bf16, tag="asum")
                nc.gpsimd.tensor_tensor(asum[:], zaT[:], amvT[:], op=Alu.add)
                aT = spool.tile([64, TB], bf16, tag="aT")
                nc.scalar.activation(aT[:], asum[:], Act.Tanh)

                # e/a back to natural [b, dv] per group: te[:, g, 0:64]=e, 64:=a
                te = ps_sml.tile([128, S, 128], bf16, tag="sml")
                for g in range(S):
                    nc.tensor.transpose(te[:, g, 0:64],
                                        eT[:, g * 128:(g + 1) * 128],
                                        ident[0:64, 0:64])
                    nc.tensor.transpose(te[:, g, 64:128],
                                        aT[:, g * 128:(g + 1) * 128],
                                        ident[0:64, 0:64])
                ena = spool.tile([128, S, 128], bf16, tag="ena")
                nc.scalar.copy(ena[:], te[:])

                # w pair-duplicated: w2[b, g, 2m, r] = w[b, m]
                w2 = spool.tile([128, S, 128], bf16, tag="w2")
                nc.gpsimd.tensor_copy(
                    w2[:].rearrange("p s (m r) -> p s m r", r=2),
                    w_nat_all[:, t, :, :].unsqueeze(3)
                    .broadcast_to([128, S, 64, 2]))

                st["mpreT"] = mpreT
                st["ena"] = ena
                st["w2"] = w2
                return st

            def combine_open(t):
                out = opool.tile([128, S, F], bf16, tag="out")
                return out

            def combine_chunk(t, st, out, k):
                """new = mpre + w*(a - mpre*e) for chunk k (g=k//2, h=k%2)."""
                mpreT, ena, w2 = st["mpreT"], st["ena"], st["w2"]
                g, h = k // 2, k % 2
                tp = ps_tp.tile([128, 2048], bf16, tag="tp")
                for i in range(1 if _abl() == "notp" else 16):
                    c = 16 * h + i
                    nc.tensor.transpose(
                        tp[:, i * 128:(i + 1) * 128],
                        mpreT[:, c, g * 128:(g + 1) * 128], ident[:])
                tpv = tp[:].rearrange("p (m d) -> p m d", m=32)
                ebig = (ena[:, g, 0:64].unsqueeze(1)
                        .broadcast_to([128, 32, 64]))
                abig = (ena[:, g, 64:128].unsqueeze(1)
                        .broadcast_to([128, 32, 64]))
                tsb = tpool.tile([128, 2048], bf16, tag="tsb")
                tv = tsb[:].rearrange("p (m d) -> p m d", m=32)
                nc.vector.tensor_tensor(tv, tpv, ebig, op=Alu.mult)
                if _abl() != "nop23":
                    nc.vector.tensor_tensor(tv, abig, tv, op=Alu.subtract)
                    w4 = (w2[:, g, 64 * h:64 * (h + 1)]
                          .rearrange("p (m r) -> p m r", r=2)
                          .unsqueeze(2).broadcast_to([128, 32, 32, 2]))
                    t4 = tsb[:].rearrange("p (m g r) -> p m g r", m=32, r=2)
                    nc.vector.tensor_tensor(t4, t4, w4, op=Alu.mult)
                nc.vector.tensor_tensor(
                    out[:, g, 2048 * h:2048 * (h + 1)],
                    tp[:], tsb[:], op=Alu.add)
                if h == 1 and _abl() != "nodma":
                    seng = nc.scalar if _abl() == "hwdge" else nc.gpsimd
                    seng.dma_start(out_r[t, :, g], out[:, g])

            def whole():
                w_nat_all = pro.tile([128, NT, S, 64], bf16, tag="w_nat_all")
                ck_all = pro.tile([128, NT, S, DK], bf16, tag="ck_all")
                nc.sync.dma_start(ck_all[:], d_ck.ap())
                loaded = load_tile(0)
                prologue(w_nat_all, ck_all)
                st = front_pre(0, loaded)
                for cc in range(NC // 2):
                    chunk_step(0, st, cc)
                st = front_post(0, w_nat_all, st)
                # steady state: interleave combine(t) with front(t+1) at
                # chunk granularity so every engine queue has ready work.
                for t in range(NT):
                    out = combine_open(t)
                    if t + 1 < NT:
                        nxt = load_tile(t + 1)
                        # cover the memT DMA latency with combine chunks
                        combine_chunk(t, st, out, 0)
                        st_next = front_pre(t + 1, nxt)
                        done = 0
                        for k in range(1, 8):
                            n = (16 * k) // 7 - done
                            for cc in range(done, done + n):
                                chunk_step(t + 1, st_next, cc)
                            done += n
                            combine_chunk(t, st, out, k)
                        st_next = front_post(t + 1, w_nat_all, st_next)
                    else:
                        for k in range(8):
                            combine_chunk(t, st, out, k)
                        st_next = None
                    st = st_next

            if iters == 1:
                whole()
            else:
                with tc.For_i(0, iters, 1,
                              hint_engines=(mybir.EngineType.PE,
                                            mybir.EngineType.DVE,
                                            mybir.EngineType.Activation,
                                            mybir.EngineType.Pool,
                                            mybir.EngineType.SP)):
                    whole()

    nc.compile()
    return nc


def _get_nc(b_core, iters, with_bm1):
    key = (b_core, iters, with_bm1, _abl())
    if key not in _BUILD_CACHE:
        _BUILD_CACHE[key] = _build(b_core, iters, with_bm1)
    return _BUILD_CACHE[key]


def _prep_weights(inputs):
    bf = ml_dtypes.bfloat16
    wc0 = np.ascontiguousarray(
        inputs["Wc0"].reshape(32, 128, 128).transpose(1, 0, 2).reshape(128, -1)
    ).astype(bf)
    wez_full = np.concatenate([inputs["Wemv"], inputs["Wzmv"]], axis=1)
    wez = np.ascontiguousarray(
        wez_full.reshape(32, 128, 128).transpose(1, 0, 2).reshape(128, -1)
    ).astype(bf)
    wamv = np.ascontiguousarray(
        inputs["Wamv"].reshape(32, 128, 64).transpose(1, 0, 2).reshape(128, -1)
    ).astype(bf)
    wewz = np.concatenate([inputs["We"], inputs["Wz"]], axis=1).astype(bf)
    wm1 = inputs["Wm1"].astype(bf)
    wza = inputs["Wza"].astype(bf)
    mkt = np.ascontiguousarray(inputs["memory_key"].T).astype(bf)

    biasv = np.zeros((128, 8), np.float32)
    biasv[:, 0] = inputs["bc0"]
    biasv[0:64, 1] = inputs["be"]
    biasv[0:64, 2] = inputs["bz"]
    biasv[0:64, 3] = inputs["bemv"]
    biasv[0:64, 4] = inputs["bzmv"]
    biasv[0:64, 5] = inputs["bamv"]
    biasv[0:64, 6] = inputs["bza"]

    w = dict(wc0=wc0, wm1=wm1, wez=wez, wamv=wamv, wewz=wewz, wza=wza,
             mkt=mkt, biasv=biasv)
    with_bm1 = bool(np.any(inputs["bm1"]))
    if with_bm1:
        w["bm1r"] = inputs["bm1"].reshape(1, F).astype(bf)
    return w, with_bm1


def _make_in_maps(inputs, b_core):
    bf = ml_dtypes.bfloat16
    wdict, _ = _prep_weights(inputs)
    nt = b_core // TB
    # memT[core, t, p, c, b] = mem[core, t*TB + b, 128c + p]
    memT = np.ascontiguousarray(
        inputs["memory_value"].reshape(N_CORES, nt, TB, NC, 128)
        .transpose(0, 1, 4, 3, 2)).astype(bf)
    # qa[core, t, p, s, f] = qa_nat[core, (t*S + s)*128 + p, f]
    qa = np.ascontiguousarray(
        inputs["control_qa"].reshape(N_CORES, nt, S, 128, DQA)
        .transpose(0, 1, 3, 2, 4)).astype(bf)
    # ck[core, p, t, s, f]
    ck = np.ascontiguousarray(
        inputs["control_key"].reshape(N_CORES, nt, S, 128, DK)
        .transpose(0, 3, 1, 2, 4)).astype(bf)
    in_maps = []
    for c in range(N_CORES):
        in_maps.append(dict(memT=memT[c], qa=qa[c], ck=ck[c], **wdict))
    return in_maps


def kernel(**inputs):
    from concourse import bass_utils
    inputs = {k: np.asarray(v) for k, v in inputs.items()}
    _, with_bm1 = _prep_weights(inputs)
    nc = _get_nc(B_CORE, 1, with_bm1)
    in_maps = _make_in_maps(inputs, B_CORE)
    res = bass_utils.run_bass_kernel_spmd(nc, in_maps, core_ids=list(range(N_CORES)))
    out = np.concatenate([r["out"] for r in res.results], axis=0)
    return out.reshape(B, M, DV).astype(np.float32)
